# revision 1
# baseline (speedup 1.0000x reference)
"""Trainium2 Bass kernel for nn_Attention_72438918414643.

Full attention block: qkv = x @ W_qkv; RMSNorm(q), RMSNorm(k); RoPE(q, k);
softmax(q k^T / sqrt(D)) v; out = o @ W_proj + b_proj.
Shapes: B=4, S=1024, C=2048, H=16, D=128.

Sharding across 8 NeuronCores: core = 2*b + g  (b = batch 0..3, g = head-group
0..1, 8 heads each).  Each core computes qkv for its (batch, head-group) slice,
full attention for its 8 heads, and a partial output projection (contraction
over its 1024 o-features).  Host sums the two partials per batch and adds
b_proj.

Device-side layout tricks:
- x is fed pre-transposed (xT [C, S]) so the QKV matmul contracts C on the
  partition dim, producing qkv in [token, feature] layout.
- q/k weight columns are permuted per head to [even d | odd d] so RoPE's
  (real, imag) pairs become contiguous 64-wide blocks; the q.k dot product is
  invariant under the (shared) permutation.
- RMSNorm weight and rope cos/sin are folded on the host into 4 per-token
  tables per q/k (exact algebra, elementwise on [S, 64]).
- After norm+rope in token layout, q/k are PE-transposed per head to [D, S]
  so scores^T = kT_chunk.T @ qT comes out with t_k on partitions; softmax is
  then exp (no max subtraction needed: |q|=|k|=sqrt(D) after RMSNorm bounds
  the logits by sqrt(D)=11.3) + a ones-matmul partition sum for Z, with the
  1/Z scaling applied to the AV output (it commutes through the linear AV).
- AV uses v chunks in natural [token, d] layout as the stationary operand,
  producing o^T [d, t] directly, which is exactly the lhsT layout the output
  projection wants; proj emits out in natural [t, c] layout for contiguous
  DMA.
All matmuls run in fp32r (TF32-like, 4x faster than fp32 on the PE).
"""

import os
import sys
import time

for _p in ("/opt/trn_rl_repo", "/root/.axon_site/_ro/trn_rl_repo"):
    if os.path.isdir(_p) and _p not in sys.path:
        sys.path.insert(0, _p)

import numpy as np

import concourse.bass as bass
import concourse.mybir as mybir
import concourse.tile as tile
from concourse import bacc
from concourse.masks import make_identity

P = 128
B = 4
S = 1024
C = 2048
H = 16
D = 128
HG = H // 2          # heads per core
TT = S // P          # token tiles
CT = C // P          # contraction tiles for qkv
FQ = 3 * HG * D      # qkv features per core (3072)
NF = FQ // 512       # qkv f-tiles
EPS = 1e-6
SMSCALE = float(D) ** -0.5
N_CORES = 8
_PHASES = int(os.environ.get("K_PHASES", "4"))
_P2 = int(os.environ.get("K_P2", "6"))

f32 = mybir.dt.float32
f32r = mybir.dt.float32r
AF = mybir.ActivationFunctionType
ALU = mybir.AluOpType
AX = mybir.AxisListType


def build_module(n_iters=1):
    nc = bacc.Bacc(None, target_bir_lowering=False, debug=False)

    xT_d = nc.dram_tensor("xT", [C, S], f32r, kind="ExternalInput")
    wq_d = nc.dram_tensor("wqkv", [C, FQ], f32r, kind="ExternalInput")
    wp_d = nc.dram_tensor("wproj", [HG * D, C], f32r, kind="ExternalInput")
    rp_d = nc.dram_tensor("rope", [S, 8 * 64], f32, kind="ExternalInput")
    out_d = nc.dram_tensor("out", [S, C], f32, kind="ExternalOutput")

    with tile.TileContext(nc) as tc:
        for _it in range(n_iters):
            # long-lived pools, managed manually (two LIFO stacks: left & right)
            constp = tc.alloc_tile_pool(name="const", bufs=1)
            persL = tc.alloc_tile_pool(name="persL", bufs=1)

            ident = constp.tile([P, P], f32)
            make_identity(nc, ident[:])
            ident_r = constp.tile([P, P], f32r)
            nc.vector.tensor_copy(ident_r[:], ident[:])
            onesf = constp.tile([P, 1], f32)
            nc.any.memset(onesf[:], 1.0)
            ones_z = constp.tile([P, 1], f32r)
            nc.vector.tensor_copy(ones_z[:], onesf[:])
            eps_t = constp.tile([P, 1], f32)
            nc.any.memset(eps_t[:], EPS)

            v_sb = persL.tile([P, TT, HG * D], f32r)

            qkp = tc.alloc_tile_pool(name="qk", bufs=1)  # q/k in token layout, ph1-2
            q_sb = qkp.tile([P, TT, HG * D], f32)
            k_sb = qkp.tile([P, TT, HG * D], f32)

            # ---------------- phase 1: QKV projection ----------------
            with (
                tc.tile_pool(name="xsb", bufs=1) as xp,
                tc.tile_pool(name="wstream", bufs=4) as wsp,
                tc.tile_pool(name="qkvps", bufs=8, space="PSUM") as pq,
            ):
                xT_sb = xp.tile([P, CT, S], f32r)
                for ct in range(CT if _PHASES >= 1 else 0):
                    nc.sync.dma_start(xT_sb[:, ct, :], xT_d[ct * P : (ct + 1) * P, :])
                for fi in range(NF if _PHASES >= 1 else 0):
                    psums = [
                        pq.tile([P, 512], f32, tag="qkvps", name=f"qkvps{fi}_{_t}")
                        for _t in range(TT)
                    ]
                    for ct in range(CT):
                        wt = wsp.tile([P, 512], f32r, tag="w")
                        nc.sync.dma_start(
                            wt[:], wq_d[ct * P : (ct + 1) * P, fi * 512 : (fi + 1) * 512]
                        )
                        for tt in range(TT):
                            nc.tensor.matmul(
                                psums[tt][:],
                                xT_sb[:, ct, tt * P : (tt + 1) * P],
                                wt[:],
                                start=(ct == 0),
                                stop=(ct == CT - 1),
                            )
                    blk, off = fi // 2, (fi % 2) * 512
                    dsts = [q_sb, k_sb, v_sb][blk]
                    for tt in range(TT):
                        nc.any.tensor_copy(dsts[:, tt, off : off + 512], psums[tt][:])

            # ---------------- phase 2: RMSNorm + RoPE + transpose -----
            # qT/kT live ph2-3 on the RIGHT stack so qk (left) can release first
            qkTp = tc.alloc_tile_pool(name="qkT", bufs=1, side="right")
            qT = qkTp.tile([P, HG, S], f32r)
            kT = qkTp.tile([P, HG, S], f32r)

            with (
                tc.tile_pool(name="ropec", bufs=1) as rcp,
                tc.tile_pool(name="rtmp", bufs=2) as rtp,
                tc.tile_pool(name="stg", bufs=2) as stp,
                tc.tile_pool(name="tps", bufs=4, space="PSUM") as tpp,
            ):
                rope_sb = rcp.tile([P, TT, 8 * 64], f32)
                nc.sync.dma_start(rope_sb[:], rp_d.rearrange("(tt p) f -> p tt f", p=P))
                for tt in range(TT if _PHASES >= 2 else 0):
                    for src, aoff, wT in ((q_sb, 0, qT), (k_sb, 256, kT)):
                        blk = src[:, tt, :]
                        b3 = blk.rearrange("p (h d) -> p h d", d=D)
                        if _P2 < 2:
                            continue
                        ssum = rtp.tile([P, HG], f32, tag="ssum")
                        sqs = rtp.tile([P, D], f32, tag="sqs")
                        for h in range(HG):
                            nc.scalar.activation(
                                sqs[:],
                                b3[:, h, :],
                                AF.Square,
                                accum_out=ssum[:, h : h + 1],
                            )
                        if _P2 < 3:
                            continue
                        srt = rtp.tile([P, HG], f32, tag="srt")
                        nc.scalar.activation(
                            srt[:], ssum[:], AF.Sqrt, scale=1.0 / D, bias=eps_t[:]
                        )
                        rs = rtp.tile([P, HG], f32, tag="rs")
                        nc.vector.reciprocal(rs[:], srt[:])
                        if _P2 < 4:
                            continue
                        th = stp.tile([P, HG, D], f32r, tag="th")
                        nc.vector.tensor_mul(
                            th[:], b3, rs[:, :, None].to_broadcast((P, HG, D))
                        )
                        thr, thi = th[:, :, 0:64], th[:, :, 64:D]

                        def tab(j):
                            lo = aoff + j * 64
                            return rope_sb[:, tt, lo : lo + 64][:, None, :].to_broadcast(
                                (P, HG, 64)
                            )

                        if _P2 < 5:
                            continue
                        m1 = rtp.tile([P, HG, 64], f32, tag="m1")
                        nc.vector.tensor_mul(m1[:], thr, tab(0))
                        m2 = rtp.tile([P, HG, 64], f32, tag="m2")
                        nc.vector.tensor_mul(m2[:], thi, tab(1))
                        m3 = rtp.tile([P, HG, 64], f32, tag="m3")
                        nc.vector.tensor_mul(m3[:], thr, tab(2))
                        m4 = rtp.tile([P, HG, 64], f32, tag="m4")
                        nc.vector.tensor_mul(m4[:], thi, tab(3))
                        nc.vector.tensor_sub(thr, m1[:], m2[:])
                        nc.vector.tensor_add(thi, m3[:], m4[:])
                        if _P2 < 6:
                            continue
                        for h in range(HG):
                            ptile = tpp.tile([P, P], f32r, tag="tp")
                            nc.tensor.transpose(ptile[:], th[:, h, :], ident_r[:])
                            nc.any.tensor_copy(wT[:, h, tt * P : (tt + 1) * P], ptile[:])

            qkp.release()  # q_sb/k_sb dead; frees 64KB/part on the left stack

            # ------------- phase 3: attention (per head) -------------
            persO = tc.alloc_tile_pool(name="persO", bufs=1)
            oT = persO.tile([P, HG, S], f32r)

            with (
                tc.tile_pool(name="pt", bufs=1) as ptp_,
                tc.tile_pool(name="zrep", bufs=2) as zrp,
                tc.tile_pool(name="rzsb", bufs=2) as rzp,
                tc.tile_pool(name="sps", bufs=4, space="PSUM") as aps,
                tc.tile_pool(name="zps", bufs=2, space="PSUM") as zps,
                tc.tile_pool(name="ops", bufs=2, space="PSUM") as ops_,
            ):
                PT = ptp_.tile([P, TT, S], f32r)
                for h in range(HG if _PHASES >= 3 else 0):
                    for tk in range(TT):
                        for tqh in range(2):
                            pss = aps.tile([P, 512], f32, tag="s")
                            nc.tensor.matmul(
                                pss[:],
                                kT[:, h, tk * P : (tk + 1) * P],
                                qT[:, h, tqh * 512 : (tqh + 1) * 512],
                                start=True,
                                stop=True,
                            )
                            nc.scalar.activation(
                                PT[:, tk, tqh * 512 : (tqh + 1) * 512],
                                pss[:],
                                AF.Exp,
                                scale=SMSCALE,
                            )
                    pz = [
                        zps.tile([1, 512], f32, tag="z", name=f"z{h}_{_t}")
                        for _t in range(2)
                    ]
                    for tk in range(TT):
                        for tqh in range(2):
                            nc.tensor.matmul(
                                pz[tqh][:],
                                ones_z[:],
                                PT[:, tk, tqh * 512 : (tqh + 1) * 512],
                                start=(tk == 0),
                                stop=(tk == TT - 1),
                            )
                    rz = rzp.tile([1, S], f32, tag="rz")
                    nc.vector.reciprocal(rz[:, 0:512], pz[0][:])
                    nc.vector.reciprocal(rz[:, 512:S], pz[1][:])
                    zrep = zrp.tile([P, S], f32, tag="zrep")
                    nc.gpsimd.partition_broadcast(zrep[:], rz[:])
                    for tqh in range(2):
                        po = ops_.tile([P, 512], f32, tag="o")
                        for tk in range(TT):
                            nc.tensor.matmul(
                                po[:],
                                v_sb[:, tk, h * D : (h + 1) * D],
                                PT[:, tk, tqh * 512 : (tqh + 1) * 512],
                                start=(tk == 0),
                                stop=(tk == TT - 1),
                            )
                        nc.vector.tensor_mul(
                            oT[:, h, tqh * 512 : (tqh + 1) * 512],
                            po[:],
                            zrep[:, tqh * 512 : (tqh + 1) * 512],
                        )

            qkTp.release()  # qT/kT dead; frees the right stack for wp/ostg

            # ---------------- phase 4: output projection ---------
            with (
                tc.tile_pool(name="wp", bufs=2, side="right") as wpp,
                tc.tile_pool(name="ostg", bufs=3, side="right") as osp,
                tc.tile_pool(name="pjps", bufs=4, space="PSUM") as pjp,
            ):
                for co in range(4 if _PHASES >= 4 else 0):
                    wpt = wpp.tile([P, HG, 512], f32r, tag="wp")
                    nc.sync.dma_start(
                        wpt[:],
                        wp_d[:, co * 512 : (co + 1) * 512].rearrange(
                            "(ci p) n -> p ci n", p=P
                        ),
                    )
                    for tt in range(TT):
                        pp = pjp.tile([P, 512], f32, tag="pj")
                        for ci in range(HG):
                            nc.tensor.matmul(
                                pp[:],
                                oT[:, ci, tt * P : (tt + 1) * P],
                                wpt[:, ci, :],
                                start=(ci == 0),
                                stop=(ci == HG - 1),
                            )
                        ost = osp.tile([P, 512], f32, tag="ost")
                        nc.any.tensor_copy(ost[:], pp[:])
                        nc.sync.dma_start(
                            out_d[tt * P : (tt + 1) * P, co * 512 : (co + 1) * 512],
                            ost[:],
                        )

            persO.release()
            persL.release()
            constp.release()
    nc.compile()
    return nc


# ------------------------- host-side preparation -------------------------


def prep_inputs(x, W_qkv, q_norm_w, k_norm_w, W_proj, b_proj, freq_cos, freq_sin):
    """Build the 8 per-core input maps."""
    x = np.asarray(x, np.float32)
    W_qkv = np.asarray(W_qkv, np.float32)
    q_norm_w = np.asarray(q_norm_w, np.float32)
    k_norm_w = np.asarray(k_norm_w, np.float32)
    W_proj = np.asarray(W_proj, np.float32)
    freq_cos = np.asarray(freq_cos, np.float32)
    freq_sin = np.asarray(freq_sin, np.float32)

    perm_d = np.concatenate([np.arange(0, D, 2), np.arange(1, D, 2)])
    wq_parts = []
    wp_parts = []
    for g in range(2):
        cols = []
        heads = range(g * HG, (g + 1) * HG)
        for h in heads:
            cols.append(h * D + perm_d)
        for h in heads:
            cols.append(C + h * D + perm_d)
        for h in heads:
            cols.append(2 * C + h * D + np.arange(D))
        wq_parts.append(np.ascontiguousarray(W_qkv[:, np.concatenate(cols)]))
        wp_parts.append(
            np.ascontiguousarray(W_proj[g * HG * D : (g + 1) * HG * D, :])
        )

    qw_r, qw_i = q_norm_w[0::2], q_norm_w[1::2]
    kw_r, kw_i = k_norm_w[0::2], k_norm_w[1::2]

    in_maps = []
    for core in range(N_CORES):
        b, g = core // 2, core % 2
        cb, sb = freq_cos[b], freq_sin[b]
        rope = np.concatenate(
            [
                cb * qw_r, sb * qw_i, sb * qw_r, cb * qw_i,
                cb * kw_r, sb * kw_i, sb * kw_r, cb * kw_i,
            ],
            axis=1,
        ).astype(np.float32)
        in_maps.append(
            {
                "xT": np.ascontiguousarray(x[b].T),
                "wqkv": wq_parts[g],
                "wproj": wp_parts[g],
                "rope": np.ascontiguousarray(rope),
            }
        )
    return in_maps


def combine_outputs(results, b_proj):
    b_proj = np.asarray(b_proj, np.float32)
    out = np.empty((B, S, C), np.float32)
    for b in range(B):
        out[b] = results[2 * b]["out"] + results[2 * b + 1]["out"] + b_proj
    return out


# ------------------------- cached PJRT runner -------------------------

_CACHE = {}


def _get_runner(n_iters=1):
    """Build (once per n_iters) a jitted shard_map executable for the module."""
    key = ("runner", n_iters)
    if key in _CACHE:
        return _CACHE[key]

    import jax
    from jax.experimental.shard_map import shard_map
    from jax.sharding import Mesh, PartitionSpec

    from concourse import bass2jax

    nc = build_module(n_iters)
    bass2jax.install_neuronx_cc_hook()

    partition_name = (
        nc.partition_id_tensor.name if nc.partition_id_tensor else None
    )
    in_names, out_names, out_avals = [], [], []
    for alloc in nc.m.functions[0].allocations:
        if not isinstance(alloc, mybir.MemoryLocationSet):
            continue
        name = alloc.memorylocations[0].name
        if alloc.kind == "ExternalInput":
            if name != partition_name:
                in_names.append(name)
        elif alloc.kind == "ExternalOutput":
            out_names.append(name)
            out_avals.append(
                jax.core.ShapedArray(
                    tuple(alloc.tensor_shape), mybir.dt.np(alloc.dtype)
                )
            )
    n_params = len(in_names)
    n_outs = len(out_names)
    all_names = in_names + out_names
    if partition_name is not None:
        all_names = all_names + [partition_name]

    def _body(*args):
        operands = list(args)
        if partition_name is not None:
            operands.append(bass2jax.partition_id_tensor())
        outs = bass2jax._bass_exec_p.bind(
            *operands,
            out_avals=tuple(out_avals),
            in_names=tuple(all_names),
            out_names=tuple(out_names),
            lowering_input_output_aliases=(),
            sim_require_finite=True,
            sim_require_nnan=True,
            nc=nc,
        )
        return tuple(outs)

    devices = jax.devices()[:N_CORES]
    mesh = Mesh(np.asarray(devices), ("core",))
    donate = tuple(range(n_params, n_params + n_outs))
    sharded = jax.jit(
        shard_map(
            _body,
            mesh=mesh,
            in_specs=(PartitionSpec("core"),) * (n_params + n_outs),
            out_specs=(PartitionSpec("core"),) * n_outs,
            check_rep=False,
        ),
        donate_argnums=donate,
        keep_unused=True,
    )

    from jax.sharding import NamedSharding

    sharding = NamedSharding(mesh, PartitionSpec("core"))

    # no-donation variant: zeros stay device-resident and are reused, so a
    # timed call transfers nothing host<->device
    sharded_nodonate = jax.jit(
        shard_map(
            _body,
            mesh=mesh,
            in_specs=(PartitionSpec("core"),) * (n_params + n_outs),
            out_specs=(PartitionSpec("core"),) * n_outs,
            check_rep=False,
        ),
        keep_unused=True,
    )

    def prep_zeros():
        return [
            jax.device_put(
                np.zeros((N_CORES * av.shape[0], *av.shape[1:]), av.dtype), sharding
            )
            for av in out_avals
        ]

    def run_timed(dev_in, dev_zeros):
        out_arrs = sharded_nodonate(*dev_in, *dev_zeros)
        for a in out_arrs:
            a.block_until_ready()

    def prep_device(in_maps):
        """Upload per-core inputs once; returns device-resident args."""
        concat_in = [
            np.concatenate([np.asarray(m[name]) for m in in_maps], axis=0)
            for name in in_names
        ]
        return [jax.device_put(a, sharding) for a in concat_in]

    def run_dev(dev_in, want_outputs=True):
        concat_zeros = [
            np.zeros((N_CORES * av.shape[0], *av.shape[1:]), av.dtype)
            for av in out_avals
        ]
        out_arrs = sharded(*dev_in, *concat_zeros)
        for a in out_arrs:
            a.block_until_ready()
        if not want_outputs:
            return None
        out_np = [np.asarray(a) for a in out_arrs]
        return [
            {
                name: out_np[i].reshape(N_CORES, *out_avals[i].shape)[c]
                for i, name in enumerate(out_names)
            }
            for c in range(N_CORES)
        ]

    def run(in_maps):
        return run_dev(prep_device(in_maps))

    _CACHE[key] = (run, prep_device, run_dev, run_timed, prep_zeros)
    return _CACHE[key]


def kernel(**inputs):
    run = _get_runner()[0]
    in_maps = prep_inputs(**{k: inputs[k] for k in (
        "x", "W_qkv", "q_norm_w", "k_norm_w", "W_proj", "b_proj",
        "freq_cos", "freq_sin")})
    results = run(in_maps)
    return combine_outputs(results, inputs["b_proj"])


# ------------------------- CoreSim helper (for test.py) -------------------------


def sim_one_core(in_map):
    """Run one core's inputs through CoreSim; returns the 'out' array."""
    from concourse.bass_interp import CoreSim

    nc = build_module()
    sim = CoreSim(nc)
    for k, v in in_map.items():
        sim.tensor(k)[:] = v
    sim.simulate()
    return np.array(sim.tensor("out"))



# revision 7
# speedup vs baseline: 3.0442x; 3.0442x over previous
"""Trainium2 Bass kernel for nn_Attention_72438918414643.

Full attention block: qkv = x @ W_qkv; RMSNorm(q), RMSNorm(k); RoPE(q, k);
softmax(q k^T / sqrt(D)) v; out = o @ W_proj + b_proj.
Shapes: B=4, S=1024, C=2048, H=16, D=128.

Sharding across 8 NeuronCores: core = 2*b + g  (b = batch 0..3, g = head-group
0..1, 8 heads each).  Each core computes qkv for its (batch, head-group) slice,
full attention for its 8 heads, and a partial output projection (contraction
over its 1024 o-features).  Host sums the two partials per batch and adds
b_proj.

Device-side layout tricks:
- x is fed pre-transposed (xT [C, S]) so the QKV matmul contracts C on the
  partition dim, producing qkv in [token, feature] layout.
- q/k weight columns are permuted per head to [even d | odd d] so RoPE's
  (real, imag) pairs become contiguous 64-wide blocks; the q.k dot product is
  invariant under the (shared) permutation.
- RMSNorm weight and rope cos/sin are folded on the host into 4 per-token
  tables per q/k (exact algebra, elementwise on [S, 64]).
- After norm+rope in token layout, q/k are PE-transposed per head to [D, S]
  so scores^T = kT_chunk.T @ qT comes out with t_k on partitions; softmax is
  then exp (no max subtraction needed: |q|=|k|=sqrt(D) after RMSNorm bounds
  the logits by sqrt(D)=11.3) + a ones-matmul partition sum for Z, with the
  1/Z scaling applied to the AV output (it commutes through the linear AV).
- AV uses v chunks in natural [token, d] layout as the stationary operand,
  producing o^T [d, t] directly, which is exactly the lhsT layout the output
  projection wants; proj emits out in natural [t, c] layout for contiguous
  DMA.
All matmuls run in fp32r (TF32-like, 4x faster than fp32 on the PE).
"""

import os
import sys
import time

for _p in ("/opt/trn_rl_repo", "/root/.axon_site/_ro/trn_rl_repo"):
    if os.path.isdir(_p) and _p not in sys.path:
        sys.path.insert(0, _p)

import numpy as np

import concourse.bass as bass
import concourse.mybir as mybir
import concourse.tile as tile
from concourse import bacc
from concourse.masks import make_identity

P = 128
B = 4
S = 1024
C = 2048
H = 16
D = 128
HG = H // 2          # heads per core
TT = S // P          # token tiles
CT = C // P          # contraction tiles for qkv
FQ = 3 * HG * D      # qkv features per core (3072)
NF = FQ // 512       # qkv f-tiles
EPS = 1e-6
SMSCALE = float(D) ** -0.5
N_CORES = 8
_PHASES = int(os.environ.get("K_PHASES", "4"))
_P2 = int(os.environ.get("K_P2", "6"))

f32 = mybir.dt.float32
f32r = mybir.dt.float32r
AF = mybir.ActivationFunctionType
ALU = mybir.AluOpType
AX = mybir.AxisListType


def build_module(n_iters=1):
    nc = bacc.Bacc(None, target_bir_lowering=False, debug=False)

    xT_d = nc.dram_tensor("xT", [C, S], f32r, kind="ExternalInput")
    wq_d = nc.dram_tensor("wqkv", [C, FQ], f32r, kind="ExternalInput")
    wp_d = nc.dram_tensor("wproj", [HG * D, C], f32r, kind="ExternalInput")
    rp_d = nc.dram_tensor("rope", [S, 8 * 64], f32, kind="ExternalInput")
    out_d = nc.dram_tensor("out", [S, C], f32, kind="ExternalOutput")

    with tile.TileContext(nc) as tc:
        for _it in range(n_iters):
            # long-lived pools, managed manually (two LIFO stacks: left & right)
            constp = tc.alloc_tile_pool(name="const", bufs=1)
            persL = tc.alloc_tile_pool(name="persL", bufs=1)

            ident = constp.tile([P, P], f32)
            make_identity(nc, ident[:])
            ident_r = constp.tile([P, P], f32r)
            nc.vector.tensor_copy(ident_r[:], ident[:])
            onesf = constp.tile([P, 1], f32)
            nc.any.memset(onesf[:], 1.0)
            ones_z = constp.tile([P, 1], f32r)
            nc.vector.tensor_copy(ones_z[:], onesf[:])
            eps_t = constp.tile([P, 1], f32)
            nc.any.memset(eps_t[:], EPS)

            v_sb = persL.tile([P, TT, HG * D], f32r)

            qkp = tc.alloc_tile_pool(name="qk", bufs=1)  # q/k in token layout, ph1-2
            q_sb = qkp.tile([P, TT, HG * D], f32)
            k_sb = qkp.tile([P, TT, HG * D], f32)

            # ---------------- phase 1: QKV projection ----------------
            with (
                tc.tile_pool(name="xsb", bufs=1) as xp,
                tc.tile_pool(name="wstream", bufs=4) as wsp,
                tc.tile_pool(name="qkvps", bufs=8, space="PSUM") as pq,
            ):
                xT_sb = xp.tile([P, CT, S], f32r)
                for ct in range(CT if _PHASES >= 1 else 0):
                    nc.sync.dma_start(xT_sb[:, ct, :], xT_d[ct * P : (ct + 1) * P, :])
                for fi in range(NF if _PHASES >= 1 else 0):
                    psums = [
                        pq.tile([P, 512], f32, tag="qkvps", name=f"qkvps{fi}_{_t}")
                        for _t in range(TT)
                    ]
                    for ct in range(CT):
                        wt = wsp.tile([P, 512], f32r, tag="w")
                        nc.sync.dma_start(
                            wt[:], wq_d[ct * P : (ct + 1) * P, fi * 512 : (fi + 1) * 512]
                        )
                        for tt in range(TT):
                            nc.tensor.matmul(
                                psums[tt][:],
                                xT_sb[:, ct, tt * P : (tt + 1) * P],
                                wt[:],
                                start=(ct == 0),
                                stop=(ct == CT - 1),
                            )
                    blk, off = fi // 2, (fi % 2) * 512
                    dsts = [q_sb, k_sb, v_sb][blk]
                    for tt in range(TT):
                        nc.any.tensor_copy(dsts[:, tt, off : off + 512], psums[tt][:])

            # ---------------- phase 2: RMSNorm + RoPE + transpose -----
            # qT/kT live ph2-3 on the RIGHT stack so qk (left) can release first
            qkTp = tc.alloc_tile_pool(name="qkT", bufs=1, side="right")
            qT = qkTp.tile([P, HG, S], f32r)
            kT = qkTp.tile([P, HG, S], f32r)

            with (
                tc.tile_pool(name="ropec", bufs=1) as rcp,
                tc.tile_pool(name="rtmp", bufs=2) as rtp,
                tc.tile_pool(name="stg", bufs=2) as stp,
                tc.tile_pool(name="tps", bufs=4, space="PSUM") as tpp,
            ):
                rope_sb = rcp.tile([P, TT, 8 * 64], f32)
                nc.sync.dma_start(rope_sb[:], rp_d.rearrange("(tt p) f -> p tt f", p=P))
                for tt in range(TT if _PHASES >= 2 else 0):
                    for src, aoff, wT in ((q_sb, 0, qT), (k_sb, 256, kT)):
                        blk = src[:, tt, :]
                        b3 = blk.rearrange("p (h d) -> p h d", d=D)
                        if _P2 < 2:
                            continue
                        ssum = rtp.tile([P, HG], f32, tag="ssum")
                        sqs = rtp.tile([P, D], f32, tag="sqs")
                        for h in range(HG):
                            nc.scalar.activation(
                                sqs[:],
                                b3[:, h, :],
                                AF.Square,
                                accum_out=ssum[:, h : h + 1],
                            )
                        if _P2 < 3:
                            continue
                        srt = rtp.tile([P, HG], f32, tag="srt")
                        nc.scalar.activation(
                            srt[:], ssum[:], AF.Sqrt, scale=1.0 / D, bias=eps_t[:]
                        )
                        rs = rtp.tile([P, HG], f32, tag="rs")
                        nc.vector.reciprocal(rs[:], srt[:])
                        if _P2 < 4:
                            continue
                        th = stp.tile([P, HG, D], f32r, tag="th")
                        nc.vector.tensor_mul(
                            th[:], b3, rs[:, :, None].to_broadcast((P, HG, D))
                        )
                        thr, thi = th[:, :, 0:64], th[:, :, 64:D]

                        def tab(j):
                            lo = aoff + j * 64
                            return rope_sb[:, tt, lo : lo + 64][:, None, :].to_broadcast(
                                (P, HG, 64)
                            )

                        if _P2 < 5:
                            continue
                        m1 = rtp.tile([P, HG, 64], f32, tag="m1")
                        nc.vector.tensor_mul(m1[:], thr, tab(0))
                        m2 = rtp.tile([P, HG, 64], f32, tag="m2")
                        nc.vector.tensor_mul(m2[:], thi, tab(1))
                        m3 = rtp.tile([P, HG, 64], f32, tag="m3")
                        nc.vector.tensor_mul(m3[:], thr, tab(2))
                        m4 = rtp.tile([P, HG, 64], f32, tag="m4")
                        nc.vector.tensor_mul(m4[:], thi, tab(3))
                        nc.vector.tensor_sub(thr, m1[:], m2[:])
                        nc.vector.tensor_add(thi, m3[:], m4[:])
                        if _P2 < 6:
                            continue
                        for h in range(HG):
                            ptile = tpp.tile([P, P], f32r, tag="tp")
                            nc.tensor.transpose(ptile[:], th[:, h, :], ident_r[:])
                            nc.any.tensor_copy(wT[:, h, tt * P : (tt + 1) * P], ptile[:])

            qkp.release()  # q_sb/k_sb dead; frees 64KB/part on the left stack

            # ------------- phase 3: attention (per head) -------------
            persO = tc.alloc_tile_pool(name="persO", bufs=1)
            oT = persO.tile([P, HG, S], f32r)

            with (
                tc.tile_pool(name="pt", bufs=1) as ptp_,
                tc.tile_pool(name="zrep", bufs=2) as zrp,
                tc.tile_pool(name="rzsb", bufs=2) as rzp,
                tc.tile_pool(name="sps", bufs=4, space="PSUM") as aps,
                tc.tile_pool(name="zps", bufs=2, space="PSUM") as zps,
                tc.tile_pool(name="ops", bufs=2, space="PSUM") as ops_,
            ):
                PT = ptp_.tile([P, TT, S], f32r)
                for h in range(HG if _PHASES >= 3 else 0):
                    for tk in range(TT):
                        for tqh in range(2):
                            pss = aps.tile([P, 512], f32, tag="s")
                            nc.tensor.matmul(
                                pss[:],
                                kT[:, h, tk * P : (tk + 1) * P],
                                qT[:, h, tqh * 512 : (tqh + 1) * 512],
                                start=True,
                                stop=True,
                            )
                            nc.scalar.activation(
                                PT[:, tk, tqh * 512 : (tqh + 1) * 512],
                                pss[:],
                                AF.Exp,
                                scale=SMSCALE,
                            )
                    pz = [
                        zps.tile([1, 512], f32, tag="z", name=f"z{h}_{_t}")
                        for _t in range(2)
                    ]
                    for tk in range(TT):
                        for tqh in range(2):
                            nc.tensor.matmul(
                                pz[tqh][:],
                                ones_z[:],
                                PT[:, tk, tqh * 512 : (tqh + 1) * 512],
                                start=(tk == 0),
                                stop=(tk == TT - 1),
                            )
                    rz = rzp.tile([1, S], f32, tag="rz")
                    nc.vector.reciprocal(rz[:, 0:512], pz[0][:])
                    nc.vector.reciprocal(rz[:, 512:S], pz[1][:])
                    zrep = zrp.tile([P, S], f32, tag="zrep")
                    nc.gpsimd.partition_broadcast(zrep[:], rz[:])
                    for tqh in range(2):
                        po = ops_.tile([P, 512], f32, tag="o")
                        for tk in range(TT):
                            nc.tensor.matmul(
                                po[:],
                                v_sb[:, tk, h * D : (h + 1) * D],
                                PT[:, tk, tqh * 512 : (tqh + 1) * 512],
                                start=(tk == 0),
                                stop=(tk == TT - 1),
                            )
                        nc.vector.tensor_mul(
                            oT[:, h, tqh * 512 : (tqh + 1) * 512],
                            po[:],
                            zrep[:, tqh * 512 : (tqh + 1) * 512],
                        )

            qkTp.release()  # qT/kT dead; frees the right stack for wp/ostg

            # ---------------- phase 4: output projection ---------
            with (
                tc.tile_pool(name="wp", bufs=2, side="right") as wpp,
                tc.tile_pool(name="ostg", bufs=3, side="right") as osp,
                tc.tile_pool(name="pjps", bufs=4, space="PSUM") as pjp,
            ):
                for co in range(4 if _PHASES >= 4 else 0):
                    wpt = wpp.tile([P, HG, 512], f32r, tag="wp")
                    nc.sync.dma_start(
                        wpt[:],
                        wp_d[:, co * 512 : (co + 1) * 512].rearrange(
                            "(ci p) n -> p ci n", p=P
                        ),
                    )
                    for tt in range(TT):
                        pp = pjp.tile([P, 512], f32, tag="pj")
                        for ci in range(HG):
                            nc.tensor.matmul(
                                pp[:],
                                oT[:, ci, tt * P : (tt + 1) * P],
                                wpt[:, ci, :],
                                start=(ci == 0),
                                stop=(ci == HG - 1),
                            )
                        ost = osp.tile([P, 512], f32, tag="ost")
                        nc.any.tensor_copy(ost[:], pp[:])
                        nc.sync.dma_start(
                            out_d[tt * P : (tt + 1) * P, co * 512 : (co + 1) * 512],
                            ost[:],
                        )

            persO.release()
            persL.release()
            constp.release()
    nc.compile()
    return nc


# ------------------------- host-side preparation -------------------------


def prep_inputs(x, W_qkv, q_norm_w, k_norm_w, W_proj, b_proj, freq_cos, freq_sin):
    """Build the 8 per-core input maps."""
    x = np.asarray(x, np.float32)
    W_qkv = np.asarray(W_qkv, np.float32)
    q_norm_w = np.asarray(q_norm_w, np.float32)
    k_norm_w = np.asarray(k_norm_w, np.float32)
    W_proj = np.asarray(W_proj, np.float32)
    freq_cos = np.asarray(freq_cos, np.float32)
    freq_sin = np.asarray(freq_sin, np.float32)

    perm_d = np.concatenate([np.arange(0, D, 2), np.arange(1, D, 2)])
    wq_parts = []
    wp_parts = []
    for g in range(2):
        cols = []
        heads = range(g * HG, (g + 1) * HG)
        for h in heads:
            cols.append(h * D + perm_d)
        for h in heads:
            cols.append(C + h * D + perm_d)
        for h in heads:
            cols.append(2 * C + h * D + np.arange(D))
        wq_parts.append(np.ascontiguousarray(W_qkv[:, np.concatenate(cols)]))
        wp_parts.append(
            np.ascontiguousarray(W_proj[g * HG * D : (g + 1) * HG * D, :])
        )

    qw_r, qw_i = q_norm_w[0::2], q_norm_w[1::2]
    kw_r, kw_i = k_norm_w[0::2], k_norm_w[1::2]

    in_maps = []
    for core in range(N_CORES):
        b, g = core // 2, core % 2
        cb, sb = freq_cos[b], freq_sin[b]
        rope = np.concatenate(
            [
                cb * qw_r, sb * qw_i, sb * qw_r, cb * qw_i,
                cb * kw_r, sb * kw_i, sb * kw_r, cb * kw_i,
            ],
            axis=1,
        ).astype(np.float32)
        in_maps.append(
            {
                "xT": np.ascontiguousarray(x[b].T),
                "wqkv": wq_parts[g],
                "wproj": wp_parts[g],
                "rope": np.ascontiguousarray(rope),
            }
        )
    return in_maps


def combine_outputs(results, b_proj):
    b_proj = np.asarray(b_proj, np.float32)
    out = np.empty((B, S, C), np.float32)
    for b in range(B):
        out[b] = results[2 * b]["out"] + results[2 * b + 1]["out"] + b_proj
    return out


# ------------------------- cached PJRT runner -------------------------

_CACHE = {}


def _get_runner(n_iters=1):
    """Build (once per n_iters) a jitted shard_map executable for the module."""
    key = ("runner", n_iters)
    if key in _CACHE:
        return _CACHE[key]

    import jax
    from jax.experimental.shard_map import shard_map
    from jax.sharding import Mesh, PartitionSpec

    from concourse import bass2jax

    nc = build_module(n_iters)
    bass2jax.install_neuronx_cc_hook()

    partition_name = (
        nc.partition_id_tensor.name if nc.partition_id_tensor else None
    )
    in_names, out_names, out_avals = [], [], []
    for alloc in nc.m.functions[0].allocations:
        if not isinstance(alloc, mybir.MemoryLocationSet):
            continue
        name = alloc.memorylocations[0].name
        if alloc.kind == "ExternalInput":
            if name != partition_name:
                in_names.append(name)
        elif alloc.kind == "ExternalOutput":
            out_names.append(name)
            out_avals.append(
                jax.core.ShapedArray(
                    tuple(alloc.tensor_shape), mybir.dt.np(alloc.dtype)
                )
            )
    n_params = len(in_names)
    n_outs = len(out_names)
    all_names = in_names + out_names
    if partition_name is not None:
        all_names = all_names + [partition_name]

    def _body(*args):
        operands = list(args)
        if partition_name is not None:
            operands.append(bass2jax.partition_id_tensor())
        outs = bass2jax._bass_exec_p.bind(
            *operands,
            out_avals=tuple(out_avals),
            in_names=tuple(all_names),
            out_names=tuple(out_names),
            lowering_input_output_aliases=(),
            sim_require_finite=True,
            sim_require_nnan=True,
            nc=nc,
        )
        return tuple(outs)

    devices = jax.devices()[:N_CORES]
    mesh = Mesh(np.asarray(devices), ("core",))
    donate = tuple(range(n_params, n_params + n_outs))
    sharded = jax.jit(
        shard_map(
            _body,
            mesh=mesh,
            in_specs=(PartitionSpec("core"),) * (n_params + n_outs),
            out_specs=(PartitionSpec("core"),) * n_outs,
            check_rep=False,
        ),
        donate_argnums=donate,
        keep_unused=True,
    )

    from jax.sharding import NamedSharding

    sharding = NamedSharding(mesh, PartitionSpec("core"))

    # no-donation variant: zeros stay device-resident and are reused, so a
    # timed call transfers nothing host<->device
    sharded_nodonate = jax.jit(
        shard_map(
            _body,
            mesh=mesh,
            in_specs=(PartitionSpec("core"),) * (n_params + n_outs),
            out_specs=(PartitionSpec("core"),) * n_outs,
            check_rep=False,
        ),
        keep_unused=True,
    )

    def prep_zeros():
        return [
            jax.device_put(
                np.zeros((N_CORES * av.shape[0], *av.shape[1:]), av.dtype), sharding
            )
            for av in out_avals
        ]

    def run_timed(dev_in, dev_zeros):
        out_arrs = sharded_nodonate(*dev_in, *dev_zeros)
        for a in out_arrs:
            a.block_until_ready()

    def prep_device(in_maps):
        """Upload per-core inputs once; returns device-resident args."""
        concat_in = [
            np.concatenate([np.asarray(m[name]) for m in in_maps], axis=0)
            for name in in_names
        ]
        return [jax.device_put(a, sharding) for a in concat_in]

    def run_dev(dev_in, want_outputs=True):
        concat_zeros = [
            np.zeros((N_CORES * av.shape[0], *av.shape[1:]), av.dtype)
            for av in out_avals
        ]
        out_arrs = sharded(*dev_in, *concat_zeros)
        for a in out_arrs:
            a.block_until_ready()
        if not want_outputs:
            return None
        out_np = [np.asarray(a) for a in out_arrs]
        return [
            {
                name: out_np[i].reshape(N_CORES, *out_avals[i].shape)[c]
                for i, name in enumerate(out_names)
            }
            for c in range(N_CORES)
        ]

    def run(in_maps):
        return run_dev(prep_device(in_maps))

    _CACHE[key] = (run, prep_device, run_dev, run_timed, prep_zeros)
    return _CACHE[key]


def kernel(**inputs):
    run = _get_runner()[0]
    in_maps = prep_inputs(**{k: inputs[k] for k in (
        "x", "W_qkv", "q_norm_w", "k_norm_w", "W_proj", "b_proj",
        "freq_cos", "freq_sin")})
    results = run(in_maps)
    return combine_outputs(results, inputs["b_proj"])


# ------------------------- CoreSim helper (for test.py) -------------------------


def sim_one_core(in_map):
    """Run one core's inputs through CoreSim; returns the 'out' array."""
    from concourse.bass_interp import CoreSim

    nc = build_module()
    sim = CoreSim(nc)
    for k, v in in_map.items():
        sim.tensor(k)[:] = v
    sim.simulate()
    return np.array(sim.tensor("out"))
